# revision 1
# baseline (speedup 1.0000x reference)
"""AdaptiveWingLoss on 8 TRN2 NeuronCores (Bass/Tile).

Shards batch (8) across cores; each core computes the weighted loss sum over
its 68 maps of 128x128, host combines partial sums into the mean.

Math (ALPHA=2.1, OMEGA=14, THETA=0.5, EPS=1, W=10), x = ln2*(t-2.1):
  dY  = max(|p - t|, 0.004)
  rm  = sigmoid(x) = tp2/(1+tp2) = 1 - r        (tp2 = 0.5**amy)
  spn = ln(1 - rm) = ln r = -log1p(tp2) = -sp
  dm2 = -2*(dY - 0.5)   (dm2 > 0 <=> small branch)
  q2n = (t-2.1)*ln(dY) = -amy*ln(dY);  eq = exp(-q2n) = dY**amy; sS = log1p(eq)
  An  = (t-2.1)*rm;  core = dm2*An = (dY-0.5)*2*amy*rm = (dY-0.5)*a/14
  l14 = select(dm2>0, sS, core - spn)    [small-branch value is just sS]
  wfb = [3x3 binary dilation of b=[t>=0.2]]  in {0,1} (borders keep b)
  out = 140*sum((wfb+0.1)*l14)/N

Inputs load as f32; the only f32-sourced compute op is the custom ABSDIFF
DVE op producing bf16 dY = max(|p-t|, 0.004).  Everything downstream is bf16
and runs in DVE 2x/4x perf modes.

Engines: DVE does the bf16 arithmetic (2x/4x modes), ACT does the 5
transcendental passes (Sigmoid table then Ln/Exp table, phase-ordered so only
2 ACT_TABLE_LOADs are emitted), Pool does u, the PSUM thresholds and the
weighted accumulation, PE does the 3x3 OR-dilation as 3 shifted band-matrix
matmuls per 512-col slice (center matrix carries the border-row deltas).
"""

import numpy as np

import concourse.bass as bass
import concourse.tile as tile
from concourse import bacc
from concourse import mybir

F32 = mybir.dt.float32
BF16 = mybir.dt.bfloat16
AF = mybir.ActivationFunctionType
ALU = mybir.AluOpType
LN2 = 0.6931471805599453

H = 128
N_CORES = 8
N_MAPS = 68  # per core (68 landmarks x 1 batch element)
SIZES = (2, 12, 12, 12, 12, 12, 6)

_ALLOWED_TABLES = ("sigmoid_and_others", "natural_log_exp_and_others")
_patched_tables = False
_custom_ops = {}


def _register_custom_ops():
    """AWL_ABSDIFF: out = max(max(in0-in1, in1-in0), s1) - s0 = clamped |p-t|."""
    if _custom_ops:
        return _custom_ops
    from concourse import dve_ops
    from concourse.dve_spec import Spec, Src0, Src1, C0, C1, maxx, lower
    from concourse.dve_uop import DveOpSpec

    defs = [
        (
            "AWL_ABSDIFF",
            Spec(
                body=maxx(maxx(Src0 - Src1, Src1 - Src0), C1) - C0,
                reference=lambda in0, in1, s0, s1, imm2: (
                    np.maximum(
                        np.maximum(
                            in0.astype(np.float32) - in1,
                            in1.astype(np.float32) - in0,
                        ),
                        s1,
                    )
                    - s0
                ).astype(np.float32),
            ),
        ),
    ]
    for name, spec in defs:
        if name in dve_ops._SUB_OPCODE_FOR_NAME:
            _custom_ops[name] = next(o for o in dve_ops.OPS if o.name == name)
            continue
        opcode = dve_ops._CUSTOM_DVE_ROW_BASE + len(dve_ops.OPS)
        assert opcode < 0x20
        shas = {}
        for ver in ("v3", "v4"):
            ds = DveOpSpec(
                name=name, opcode=opcode, uops=lower(spec, ver=ver), rd1_en=True
            )
            shas[ver] = ds.sha(ver)
        dve_ops._SUB_OPCODE_FOR_NAME[name] = opcode
        op = dve_ops.DveOp(name, spec, subdim=False, uops_sha=shas)
        dve_ops.OPS.append(op)
        dve_ops.CUSTOM_DVE_SPECS[name] = spec
        _custom_ops[name] = op
    return _custom_ops


def _patch_act_tables():
    """Restrict bacc's activation-set choices to the two sets we phase over
    (Sigmoid; Ln+Exp) so the fixpoint pass emits exactly one load per phase."""
    global _patched_tables
    if _patched_tables:
        return
    orig = bacc.get_activation_tables

    def patched(arch):
        tabs = orig(arch)
        return {k: (v if k in _ALLOWED_TABLES else set()) for k, v in tabs.items()}

    bacc.get_activation_tables = patched
    _patched_tables = True


def make_vband():
    """[128, 256] f32 stationaries for the vertical OR:
    cols 0:128   M_side   = 3-row band, interior output rows 1..126 only
    cols 128:256 M_center = band + delta at rows 0/127 (border rows keep b)."""
    v = np.zeros((H, 2 * H), dtype=np.float32)
    for i in range(1, H - 1):
        for k in (i - 1, i, i + 1):
            v[k, i] = 1.0
            v[k, H + i] = 1.0
    v[0, H + 0] = 1.0
    v[H - 1, H + H - 1] = 1.0
    return v


def build_nc(n_maps=N_MAPS, sizes=SIZES):
    _patch_act_tables()
    ops = _register_custom_ops()
    assert sum(sizes) == n_maps
    chunks = []
    m0 = 0
    for c in sizes:
        chunks.append((m0, c))
        m0 += c
    nch = len(chunks)
    cm = max(sizes)
    FT = n_maps * H  # total cols per partition

    nc = bacc.Bacc("TRN2")
    pred = nc.declare_dram_parameter("predictions", [n_maps, H, H], F32, isOutput=False)
    targ = nc.declare_dram_parameter("targets", [n_maps, H, H], F32, isOutput=False)
    vband = nc.declare_dram_parameter("vband", [H, 2 * H], BF16, isOutput=False)
    outd = nc.declare_dram_parameter("out", [H, nch], F32, isOutput=True)

    with tile.TileContext(nc) as tc:
        with (
            tc.tile_pool(name="io", bufs=2) as iop,
            tc.tile_pool(name="wk", bufs=1) as wk,
            tc.tile_pool(name="acc", bufs=1) as accp,
            tc.tile_pool(name="psum", bufs=4, space="PSUM") as psp,
        ):
            acc = accp.tile([H, nch], F32, tag="acc", name="acc")
            bias_sg = accp.tile([H, 1], F32, tag="bias_sg", name="bias_sg")
            nc.gpsimd.memset(bias_sg[:], -2.1 * LN2)
            bias_half = accp.tile([H, 1], F32, tag="bias_half", name="bias_half")
            nc.gpsimd.memset(bias_half[:], -0.5)
            # band stationaries (bf16, converted host-side)
            vb = accp.tile([H, 2 * H], BF16, tag="vb", name="vb")
            nc.sync.dma_start(out=vb[:], in_=vband[:])
            b_gs = []
            for k in range(2):
                bg = accp.tile([H, cm * H + 4], BF16, tag=f"b_g{k}", name=f"b_g{k}")[:]
                nc.gpsimd.memset(bg, 0.0)
                b_gs.append(bg)

            # full-resolution residents (live across the two phases)
            rm_g = accp.tile([H, FT], BF16, tag="rm_g", name="rm_g")[:]
            aD_g = accp.tile([H, FT], BF16, tag="aD_g", name="aD_g")[:]
            dm_g = accp.tile([H, FT], BF16, tag="dm_g", name="dm_g")[:]
            wf_g = accp.tile([H, FT], BF16, tag="wf_g", name="wf_g")[:]
            am_g = accp.tile([H, FT], BF16, tag="am_g", name="am_g")[:]
            tt_g = accp.tile([H, FT], F32, tag="tt_g", name="tt_g")[:]

            # target loads first (2 queues), then rm per chunk
            for ci, (m0, c) in enumerate(chunks):
                F = c * H
                f0 = m0 * H
                eng = nc.sync if ci % 2 == 0 else nc.gpsimd
                eng.dma_start(
                    out=tt_g[:, f0 : f0 + F].rearrange("p (m w) -> p m w", w=H),
                    in_=targ[m0 : m0 + c].rearrange("m h w -> h m w"),
                )
            for m0, c in chunks:
                F = c * H
                f0 = m0 * H
                nc.scalar.activation(
                    rm_g[:, f0 : f0 + F], tt_g[:, f0 : f0 + F],
                    AF.Sigmoid, bias=bias_sg[:], scale=LN2,
                )

            # scheduling gates on the (idle) pool engine: their writes depend
            # on the LAST rm, so phase-2 activations (which use them as bias)
            # come after all Sigmoids -> only 2 ACT_TABLE_LOADs.
            last = rm_g[:, FT - 1 : FT]
            gate0 = accp.tile([H, 1], F32, tag="gate0", name="gate0")[:]
            gate1 = accp.tile([H, 1], F32, tag="gate1", name="gate1")[:]
            one_t = accp.tile([H, 1], F32, tag="one_t", name="one_t")
            nc.gpsimd.memset(one_t[:], 1.0)
            nc.gpsimd.tensor_tensor(gate0, last, last, ALU.subtract)
            nc.gpsimd.tensor_tensor(gate1, gate0, one_t[:], ALU.add)

            tiles = {}

            def phase1(ci, m0, c):
                F = c * H
                f0 = m0 * H
                tt = tt_g[:, f0 : f0 + F]
                tp = iop.tile([H, F], F32, tag="tp", name=f"tp{ci}")
                eng = nc.gpsimd if ci % 2 == 0 else nc.sync
                eng.dma_start(
                    out=tp[:].rearrange("p (m w) -> p m w", w=H),
                    in_=pred[m0 : m0 + c].rearrange("m h w -> h m w"),
                )
                tp = tp[:]
                aD = aD_g[:, f0 : f0 + F]
                dm2 = dm_g[:, f0 : f0 + F]
                rm = rm_g[:, f0 : f0 + F]
                amy = am_g[:, f0 : f0 + F]
                wf = wf_g[:, f0 : f0 + F]
                b_g = b_gs[ci % 2]
                b = b_g[:, 2 : F + 2]

                nc.scalar.activation(amy, tt, AF.Copy, bias=-2.1)
                nc.vector._custom_dve(
                    ops["AWL_ABSDIFF"], out=aD, in0=tp, in1=tt, s0=0.0, s1=0.004
                )
                nc.vector.tensor_scalar(dm2, aD, 0.5, -2.0, ALU.subtract, ALU.mult)
                # b = [t >= 0.2] = [rm >= sigmoid(ln2*(0.2-2.1))]
                nc.vector.tensor_scalar(b, rm, 0.2113316, None, ALU.is_ge)
                # border cols keep b: wf = 2b - 1 (independent of the interior
                # sign passes, which write disjoint columns)
                wf3 = wf.rearrange("p (m w) -> p m w", w=H)
                b3 = b.rearrange("p (m w) -> p m w", w=H)
                nc.vector.tensor_scalar(
                    wf3[:, :, 0:1], b3[:, :, 0:1], 2.0, -1.0, ALU.mult, ALU.add
                )
                nc.vector.tensor_scalar(
                    wf3[:, :, H - 1 : H], b3[:, :, H - 1 : H], 2.0, -1.0,
                    ALU.mult, ALU.add,
                )

            def phase2a(ci, m0, c):
                F = c * H
                f0 = m0 * H
                aD = aD_g[:, f0 : f0 + F]
                rm = rm_g[:, f0 : f0 + F]
                amy = am_g[:, f0 : f0 + F]
                dm2 = dm_g[:, f0 : f0 + F]

                def T(tag):
                    return wk.tile([H, F], BF16, tag=tag, name=f"{tag}{ci}", bufs=2)[:]

                lnd, spn, q2n, An, core = T("lnd"), T("spn"), T("q2n"), T("An"), T("core")
                mask = wk.tile([H, F], mybir.dt.uint16, tag="mask", name=f"mask{ci}", bufs=2)[:]
                nc.scalar.activation(lnd, aD, AF.Ln, bias=gate0)
                nc.scalar.activation(spn, rm, AF.Ln, bias=gate1, scale=-1.0)
                nc.vector.tensor_tensor(q2n, amy, lnd, ALU.mult)
                nc.vector.tensor_tensor(An, amy, rm, ALU.mult)
                nc.vector.tensor_tensor(core, dm2, An, ALU.mult)
                nc.vector.tensor_scalar(mask, dm2, 0.0, None, ALU.is_gt)
                tiles[ci] = (spn, q2n, core, mask)

            def phase2b(ci, m0, c):
                F = c * H
                f0 = m0 * H
                dm2 = dm_g[:, f0 : f0 + F]
                wf = wf_g[:, f0 : f0 + F]
                b_g = b_gs[ci % 2]
                spn, q2n, core, mask = tiles.pop(ci)

                def T(tag):
                    return wk.tile([H, F], BF16, tag=tag, name=f"{tag}{ci}", bufs=2)[:]

                # dilation: 3 col-shifted matmuls per 512-slice + Sign drain
                for c0 in range(0, F, 512):
                    cw = min(512, F - c0)
                    ps = psp.tile([H, cw], F32, tag="ps", name=f"ps{ci}_{c0}")
                    nc.tensor.matmul(
                        ps[:], vb[:, 0:H], b_g[:, c0 + 1 : c0 + 1 + cw],
                        start=True, stop=False,
                    )
                    nc.tensor.matmul(
                        ps[:], vb[:, H : 2 * H], b_g[:, c0 + 2 : c0 + 2 + cw],
                        start=False, stop=False,
                    )
                    nc.tensor.matmul(
                        ps[:], vb[:, 0:H], b_g[:, c0 + 3 : c0 + 3 + cw],
                        start=False, stop=True,
                    )
                    # interior cols only; border cols were set from b in
                    # phase 1 and must not be overwritten by dilated values
                    nc.scalar.activation(
                        wf[:, c0 : c0 + cw].rearrange("p (m w) -> p m w", w=H)[:, :, 1 : H - 1],
                        ps[:].rearrange("p (m w) -> p m w", w=H)[:, :, 1 : H - 1],
                        AF.Sign, bias=bias_half[:],
                    )


                eq, sS, l14 = T("eq"), T("sS"), T("l14")
                waste = wk.tile([H, F], BF16, tag="eq", name=f"waste{ci}", bufs=2)[:]
                nc.scalar.activation(eq, q2n, AF.Exp, scale=-1.0, bias=gate0)
                nc.scalar.activation(sS, eq, AF.Ln, bias=gate1)
                nc.vector.tensor_tensor(l14, core, spn, ALU.subtract)
                nc.vector.copy_predicated(l14, mask, sS)
                nc.vector.scalar_tensor_tensor(
                    waste, wf, 1.2, l14, ALU.add, ALU.mult,
                    accum_out=acc[:, ci : ci + 1],
                )

            # 3-deep software pipeline
            for i in range(nch + 2):
                if i < nch:
                    phase1(i, *chunks[i])
                if 1 <= i <= nch:
                    phase2a(i - 1, *chunks[i - 1])
                if i >= 2:
                    phase2b(i - 2, *chunks[i - 2])
            nc.sync.dma_start(out=outd[:], in_=acc[:])
    nc.compile()
    return nc


_TRACE = {"enabled": False, "last": None}


def kernel(predictions, targets):
    from concourse.bass_utils import run_bass_kernel_spmd

    preds = np.ascontiguousarray(predictions, dtype=np.float32)
    targs = np.ascontiguousarray(targets, dtype=np.float32)
    B = preds.shape[0]
    import ml_dtypes
    vband = make_vband().astype(ml_dtypes.bfloat16)
    in_maps = [
        {"predictions": preds[i], "targets": targs[i], "vband": vband}
        for i in range(N_CORES)
    ]
    nc = build_nc()
    kwargs = {}
    if _TRACE["enabled"]:
        kwargs = {"trace": True}
    try:
        res = run_bass_kernel_spmd(nc, in_maps, core_ids=list(range(N_CORES)), **kwargs)
    except Exception:
        if not kwargs:
            raise
        res = run_bass_kernel_spmd(nc, in_maps, core_ids=list(range(N_CORES)))
    _TRACE["last"] = res
    tot = 0.0
    for r in res.results:
        o = np.asarray(r["out"], dtype=np.float64)
        tot += 70.0 * o.sum()
    n_total = B * N_MAPS * H * H
    return np.float32(tot / n_total)

